# revision 19
# baseline (speedup 1.0000x reference)
"""Trainium2 Bass kernel for nn_CESAR_24309514895978 (ragged_sequence).

Math (per batch b):
  m0 = (attention_masks==1)&(token_type_ids==0); m1 = (attention_masks==1)&(token_type_ids==1)
  score[i,j] = |emb_n[i] . emb_n[j]|   (L2-normalized embeddings)
  logits[i,j] = (emb@Wq.T+bq)[i] . (emb@Wk.T+bk)[j]
  cs[b] = sum_{valid ij} softmax_flat(logits | pair_mask)[i,j] * score[i,j]

Constant folding (host, once): the projections only enter through
  logits = embaug @ A_aug @ embaug.T,  embaug = [emb, 1],
  A_aug = [[Wq.T@Wk, Wq.T@bk], [bq.T@Wk, bq.bk]]   ((D+1)x(D+1))
so the per-batch device work is two chained matmuls instead of three.

Device, per batch (data-parallel: 2 batches per core x 8 cores, fp32r matmuls):
  - rsq[j] = sum_d emb[j,d]^2 (DVE squares+adds, one ones-matmul); r = 1/sqrt
  - P = A_aug @ embaug.T   (stage 1, 8 PSUM banks, db-outer accumulation)
  - L = embaug.T.T @ P + (-1e30 masks via a K=2 static matmul row-pair)
  - M = masked max (DVE reduces + gpsimd partition_all_reduce)
  - E = exp(L - M) on ACT with accum_out -> Z partial sums
  - W partials = sum_j E * |G| * r_j  (G = gram matmul; |.| on ACT; stt fused)
Host: r_i scaling + final sums + W/Z division (tiny) + input layout/rounding.
"""
import numpy as np

import concourse.bass_isa as bass_isa
import concourse.tile as tile
from concourse import bacc, mybir
from concourse.bass_utils import run_bass_kernel_spmd

B, S, D = 16, 512, 1024
NCORES = 8
BPC = B // NCORES          # batches per core
NCH = D // 128             # 8 contraction chunks
NIC = S // 128             # 4 i-chunks
DA = D + 1                 # augmented dim
NEG = np.float32(-1e30)

F32 = mybir.dt.float32
F32R = mybir.dt.float32r
AFT = mybir.ActivationFunctionType
ALU = mybir.AluOpType
AX = mybir.AxisListType

PROFILE = False            # set True (e.g. from test.py) to capture NTFF profile
LAST_RESULTS = None        # BassKernelResults of the last run (for test.py)

_built = None


def _to_fp32r(x: np.ndarray) -> np.ndarray:
    """Round fp32 -> fp32r encoding (RNE to 11 explicit mantissa bits)."""
    u = np.ascontiguousarray(x, dtype=np.float32).view(np.uint32).astype(np.uint64)
    u = (u + 0x7FF + ((u >> 12) & 1)) & np.uint64(0xFFFFF000)
    return u.astype(np.uint32).view(np.float32)


def _build():
    global _built
    if _built is not None:
        return _built

    nc = bacc.Bacc("TRN2", target_bir_lowering=False, debug=False)

    embT_d = nc.dram_tensor("embT", [BPC, NCH, 128, S], F32R, kind="ExternalInput").ap()
    # AT[db, da] = A_aug[da, db]; rows 0..1023 in 8 chunks + row 1024 separate
    at_d = nc.dram_tensor("at", [DA, DA], F32R, kind="ExternalInput").ap()
    lrows_d = nc.dram_tensor("lrows", [BPC, 3, S], F32R, kind="ExternalInput").ap()
    rrows_d = nc.dram_tensor("rrows", [BPC, 2, S], F32R, kind="ExternalInput").ap()
    ucol_d = nc.dram_tensor("ucol", [128, NCH], F32, kind="ExternalInput").ap()
    c0_d = nc.dram_tensor("c0", [1, 1], F32, kind="ExternalInput").ap()
    ones_d = nc.dram_tensor("ones", [128, 1], F32R, kind="ExternalInput").ap()
    onesrow_d = nc.dram_tensor("onesrow", [1, S], F32R, kind="ExternalInput").ap()

    zw_d = nc.dram_tensor("zw", [BPC, 2, 128, NIC], F32, kind="ExternalOutput").ap()
    rout_d = nc.dram_tensor("rout", [BPC, S], F32, kind="ExternalOutput").ap()

    with tile.TileContext(nc) as tc:
        with (
            tc.tile_pool(name="apool", bufs=9) as apool,
            tc.tile_pool(name="spool", bufs=1) as spool,
            tc.tile_pool(name="epool", bufs=16) as epool,
            tc.tile_pool(name="sqpool", bufs=3) as sqpool,
            tc.tile_pool(name="paugpool", bufs=18) as paugpool,
            tc.tile_pool(name="w2pool", bufs=2) as w2pool,
            tc.tile_pool(name="gapool", bufs=2) as gapool,
            tc.tile_pool(name="gwpool", bufs=4) as gwpool,
            tc.tile_pool(name="Epool", bufs=2) as Epool,
            tc.tile_pool(name="scrpool", bufs=1) as scrpool,
            tc.tile_pool(name="tiny", bufs=2) as tiny,
            tc.tile_pool(name="lrpool", bufs=2) as lrpool,
            tc.tile_pool(name="ps", bufs=8, space="PSUM") as ps,
        ):
            # ---- first chunk pair goes absolutely first (PE start gate),
            # then the tiny loads, then the remaining big chunks interleaved.
            emb_all = [[None] * NCH for _ in range(BPC)]
            at_t = []
            t = apool.tile([128, DA], F32R, tag="a", name="at_0")
            nc.sync.dma_start(out=t[:], in_=at_d[0:128, :])
            at_t.append(t)
            t = epool.tile([128, S], F32R, tag="emb", name="emb0_0")
            nc.sync.dma_start(out=t[:], in_=embT_d[0, 0])
            emb_all[0][0] = t

            # ---- HAM warmup: dummy matmuls on the first AT chunk while the
            # rest of the inputs stream in, so the PE clock is at 2.4GHz when
            # the real work starts (cold MMs run 2x slower for ~3.4us).
            warm_ps = ps.tile([128, S], F32, tag="ps", name="warm_ps")
            for _ in range(10):
                nc.tensor.matmul(warm_ps[:], at_t[0][:, 0:128], at_t[0][:, 0:S],
                                 start=True, stop=True)

            ones_col = spool.tile([128, 1], F32R, tag="ones_col")
            nc.sync.dma_start(out=ones_col[:], in_=ones_d)
            onesrow_t = spool.tile([1, S], F32R, tag="onesrow")
            nc.sync.dma_start(out=onesrow_t[:], in_=onesrow_d)
            ucol_t = spool.tile([128, NCH], F32, tag="ucol")
            nc.sync.dma_start(out=ucol_t[:], in_=ucol_d)
            c0_t = spool.tile([1, 1], F32, tag="c0")
            nc.sync.dma_start(out=c0_t[:], in_=c0_d)
            lr_all = []
            for b in range(BPC):
                lr_t = lrpool.tile([3, S], F32R, tag="lr", name=f"lr{b}")
                nc.sync.dma_start(out=lr_t[:], in_=lrows_d[b])
                lr_all.append(lr_t)

            for c in range(1, NCH):
                t = epool.tile([128, S], F32R, tag="emb", name=f"emb0_{c}")
                nc.sync.dma_start(out=t[:], in_=embT_d[0, c])
                emb_all[0][c] = t
                t = apool.tile([128, DA], F32R, tag="a", name=f"at_{c}")
                nc.sync.dma_start(out=t[:], in_=at_d[c * 128 : (c + 1) * 128, :])
                at_t.append(t)

            for b in range(BPC):
                # ---- load this batch's emb
                if b > 0:
                    for c in range(NCH):
                        t = epool.tile([128, S], F32R, tag="emb", name=f"emb{b}_{c}")
                        nc.sync.dma_start(out=t[:], in_=embT_d[b, c])
                        emb_all[b][c] = t
                emb_t = emb_all[b]
                lr_t = lr_all[b]

                # ---- stage 1: P = A_aug @ embaug.T  (db-outer over 8 banks);
                # the ones-row term (u) is folded into the copy bias below.
                st1 = [ps.tile([128, S], F32, tag="ps", name=f"st1_{b}_{da}")
                       for da in range(NCH)]
                for db in range(NCH):
                    for da in range(NCH):
                        nc.tensor.matmul(st1[da][:],
                                         at_t[db][:, da * 128 : (da + 1) * 128],
                                         emb_t[db][:],
                                         start=(db == 0), stop=(db == NCH - 1))
                paug = []
                for da in range(NCH):
                    pt = paugpool.tile([128, S], F32R, tag="paug")
                    if da % 2 == 0:
                        nc.scalar.activation(out=pt[:], in_=st1[da][:],
                                             func=AFT.Identity,
                                             bias=ucol_t[:, da : da + 1], scale=1.0)
                    else:
                        nc.vector.tensor_scalar_add(pt[:], st1[da][:],
                                                    ucol_t[:, da : da + 1])
                    paug.append(pt)
                # P row 1024 (the bq-side rank-1 term); c0 folded into the bias
                prow_ps = ps.tile([1, S], F32, tag="ps")
                for db in range(NCH):
                    nc.tensor.matmul(prow_ps[:], at_t[db][:, D : D + 1],
                                     emb_t[db][:],
                                     start=(db == 0), stop=(db == NCH - 1))
                prow = tiny.tile([1, S], F32R, tag="prow")
                nc.scalar.activation(out=prow[:], in_=prow_ps[:],
                                     func=AFT.Identity, bias=c0_t[:], scale=1.0)
                # rhs rows for the combined mask+prow matmul (K=3):
                # p0 = ones, p1 = m1neg (host), p2 = prow (device)
                rr3 = lrpool.tile([3, S], F32R, tag="rr3")
                nc.sync.dma_start(out=rr3[0:2, :], in_=rrows_d[b])
                nc.sync.dma_start(out=rr3[2:3, :], in_=prow[:])

                # ---- rsq / r / W2
                sqacc = sqpool.tile([128, S], F32R, tag="sqacc", bufs=2)
                sq0 = sqpool.tile([128, S], F32, tag="sq")
                nc.vector.tensor_mul(sq0[:], emb_t[0][:].bitcast(F32),
                                     emb_t[0][:].bitcast(F32))
                for c in range(1, NCH):
                    sq = sqpool.tile([128, S], F32, tag="sq")
                    nc.vector.tensor_mul(sq[:], emb_t[c][:].bitcast(F32),
                                         emb_t[c][:].bitcast(F32))
                    if c < NCH - 1:
                        nc.vector.tensor_add(sq0[:], sq0[:], sq[:])
                    else:
                        nc.vector.tensor_add(sqacc[:], sq0[:], sq[:])
                rsq_ps = ps.tile([1, S], F32, tag="ps")
                nc.tensor.matmul(rsq_ps[:], ones_col[:], sqacc[:],
                                 start=True, stop=True)
                s_row = tiny.tile([1, S], F32, tag="srow")
                nc.scalar.activation(out=s_row[:], in_=rsq_ps[:], func=AFT.Sqrt,
                                     bias=0.0, scale=1.0)
                r_row = tiny.tile([1, S], F32, tag="rrow")
                nc.vector.reciprocal(out=r_row[:], in_=s_row[:])
                nc.sync.dma_start(out=rout_d[b], in_=r_row[:])
                W2 = w2pool.tile([128, S], F32, tag="w2")
                nc.gpsimd.partition_broadcast(W2[:], r_row[0:1, :], channels=128)

                # ---- stage 2: L chunks + masks; per-chunk max
                mx = tiny.tile([128, NIC], F32, tag="mx")
                L_ps = []
                for ic in range(NIC):
                    Lp = ps.tile([128, S], F32, tag="ps", name=f"L_{b}_{ic}")
                    for da in range(NCH):
                        nc.tensor.matmul(Lp[:], emb_t[da][:, ic * 128 : (ic + 1) * 128],
                                         paug[da][:], start=(da == 0), stop=False)
                    nc.tensor.matmul(Lp[:], lr_t[:, ic * 128 : (ic + 1) * 128],
                                     rr3[:], start=False, stop=True)
                    nc.vector.reduce_max(mx[:, ic : ic + 1], Lp[:], axis=AX.X)
                    L_ps.append(Lp)

                # ---- global masked max -> -M in [128,1]
                par = tiny.tile([128, NIC], F32, tag="par")
                nc.gpsimd.partition_all_reduce(par[:], mx[:], channels=128,
                                               reduce_op=bass_isa.ReduceOp.max)
                negm128 = tiny.tile([128, 1], F32, tag="negm128")
                nc.vector.reduce_max(negm128[:], par[:], axis=AX.X, negate=True)

                # ---- gram chunks -> Gw = |G| * r_j
                gw_t = []
                for ic in range(NIC):
                    Gp = ps.tile([128, S], F32, tag="ps", name=f"G_{b}_{ic}")
                    for c in range(NCH):
                        nc.tensor.matmul(Gp[:], emb_t[c][:, ic * 128 : (ic + 1) * 128],
                                         emb_t[c][:], start=(c == 0), stop=(c == NCH - 1))
                    ga = gapool.tile([128, S], F32, tag="ga")
                    nc.scalar.activation(out=ga[:], in_=Gp[:], func=AFT.Abs,
                                         bias=0.0, scale=1.0)
                    gw = gwpool.tile([128, S], F32, tag="gw")
                    nc.vector.tensor_mul(gw[:], ga[:], W2[:])
                    gw_t.append(gw)

                # ---- exp + fused weighted reductions
                zwcols = tiny.tile([128, 2 * NIC], F32, tag="zwc")
                zcols = zwcols[:, 0:NIC]
                wcols = zwcols[:, NIC : 2 * NIC]
                for ic in range(NIC):
                    E = Epool.tile([128, S], F32, tag="E")
                    nc.scalar.activation(out=E[:], in_=L_ps[ic][:], func=AFT.Exp,
                                         bias=negm128[:], scale=1.0,
                                         accum_out=zcols[:, ic : ic + 1])
                    scr = scrpool.tile([128, S], F32, tag="scr")
                    nc.vector.scalar_tensor_tensor(
                        out=scr[:], in0=gw_t[ic][:], scalar=1.0, in1=E[:],
                        op0=ALU.mult, op1=ALU.mult,
                        accum_out=wcols[:, ic : ic + 1])

                nc.sync.dma_start(out=zw_d[b, 0], in_=zcols[:])
                nc.sync.dma_start(out=zw_d[b, 1], in_=wcols[:])

    nc.compile()
    _built = nc
    return nc


def kernel(embeddings, Wq, bq, Wk, bk, attention_masks, token_type_ids):
    global LAST_RESULTS
    nc = _build()

    embeddings = np.ascontiguousarray(np.asarray(embeddings, dtype=np.float32))
    Wq = np.asarray(Wq, dtype=np.float32)
    Wk = np.asarray(Wk, dtype=np.float32)
    bq = np.asarray(bq, dtype=np.float32)
    bk = np.asarray(bk, dtype=np.float32)
    am = np.asarray(attention_masks)
    tt = np.asarray(token_type_ids)

    # host-side layout + constant folding + fp32r rounding
    embT = _to_fp32r(embeddings.transpose(0, 2, 1)).reshape(B, NCH, 128, S)

    Wq64, Wk64 = Wq.astype(np.float64), Wk.astype(np.float64)
    A_aug = np.empty((DA, DA), np.float64)
    A_aug[:D, :D] = Wq64.T @ Wk64                  # A[d,d'] = sum_e Wq[e,d] Wk[e,d']
    A_aug[:D, D] = Wq64.T @ bk.astype(np.float64)   # u
    A_aug[D, :D] = Wk64.T @ bq.astype(np.float64)   # v
    A_aug[D, D] = float(bq.astype(np.float64) @ bk.astype(np.float64))
    AT = _to_fp32r(np.ascontiguousarray(A_aug.T).astype(np.float32))

    tok = am == 1
    m0 = tok & (tt == 0)
    m1 = tok & (tt == 1)
    m0neg = np.where(m0, np.float32(0.0), NEG).astype(np.float32)
    m1neg = np.where(m1, np.float32(0.0), NEG).astype(np.float32)
    ones_row = np.ones((B, 1, S), np.float32)
    lrows = _to_fp32r(np.concatenate([m0neg[:, None, :], ones_row, ones_row], axis=1))
    rrows = _to_fp32r(np.concatenate([ones_row, m1neg[:, None, :]], axis=1))
    ucol = np.ascontiguousarray(
        A_aug[:D, D].astype(np.float32).reshape(NCH, 128).T)        # [128, NCH]
    c0 = np.array([[A_aug[D, D]]], np.float32)

    in_maps = []
    for i in range(NCORES):
        sl = slice(i * BPC, (i + 1) * BPC)
        in_maps.append({
            "embT": np.ascontiguousarray(embT[sl]),
            "at": AT,
            "lrows": np.ascontiguousarray(lrows[sl]),
            "rrows": np.ascontiguousarray(rrows[sl]),
            "ones": np.ones((128, 1), np.float32),
            "onesrow": np.ones((1, S), np.float32),
            "ucol": ucol, "c0": c0,
        })

    res = run_bass_kernel_spmd(nc, in_maps, core_ids=list(range(NCORES)),
                               trace=PROFILE)
    LAST_RESULTS = res

    valid = m0.any(axis=1) & m1.any(axis=1)
    cs = np.zeros(B, np.float64)
    for i in range(NCORES):
        for j in range(BPC):
            b = i * BPC + j
            if not valid[b]:
                continue
            zcols = res.results[i]["zw"][j, 0].astype(np.float64)   # [128, NIC]
            wcols = res.results[i]["zw"][j, 1].astype(np.float64)
            r = res.results[i]["rout"][j].astype(np.float64)        # [S]
            ri = r.reshape(NIC, 128).T                              # [128, NIC]
            z = zcols.sum()
            w = (wcols * ri).sum()
            cs[b] = w / (z + 1e-30)
    return cs.astype(np.float32)


# revision 20
# speedup vs baseline: 1.0044x; 1.0044x over previous
"""Trainium2 Bass kernel for nn_CESAR_24309514895978 (ragged_sequence).

Math (per batch b):
  m0 = (attention_masks==1)&(token_type_ids==0); m1 = (attention_masks==1)&(token_type_ids==1)
  score[i,j] = |emb_n[i] . emb_n[j]|   (L2-normalized embeddings)
  logits[i,j] = (emb@Wq.T+bq)[i] . (emb@Wk.T+bk)[j]
  cs[b] = sum_{valid ij} softmax_flat(logits | pair_mask)[i,j] * score[i,j]

Constant folding (host, once): the projections only enter through
  logits = embaug @ A_aug @ embaug.T,  embaug = [emb, 1],
  A_aug = [[Wq.T@Wk, Wq.T@bk], [bq.T@Wk, bq.bk]]   ((D+1)x(D+1))
so the per-batch device work is two chained matmuls instead of three.

Device, per batch (data-parallel: 2 batches per core x 8 cores, fp32r matmuls):
  - rsq[j] = sum_d emb[j,d]^2 (DVE squares+adds, one ones-matmul); r = 1/sqrt
  - P = A_aug @ embaug.T   (stage 1, 8 PSUM banks, db-outer accumulation)
  - L = embaug.T.T @ P + (-1e30 masks via a K=2 static matmul row-pair)
  - M = masked max (DVE reduces + gpsimd partition_all_reduce)
  - E = exp(L - M) on ACT with accum_out -> Z partial sums
  - W partials = sum_j E * |G| * r_j  (G = gram matmul; |.| on ACT; stt fused)
Host: r_i scaling + final sums + W/Z division (tiny) + input layout/rounding.
"""
import numpy as np

import concourse.bass_isa as bass_isa
import concourse.tile as tile
from concourse import bacc, mybir
from concourse.bass_utils import run_bass_kernel_spmd

B, S, D = 16, 512, 1024
NCORES = 8
BPC = B // NCORES          # batches per core
NCH = D // 128             # 8 contraction chunks
NIC = S // 128             # 4 i-chunks
DA = D + 1                 # augmented dim
NEG = np.float32(-1e30)

F32 = mybir.dt.float32
F32R = mybir.dt.float32r
AFT = mybir.ActivationFunctionType
ALU = mybir.AluOpType
AX = mybir.AxisListType

PROFILE = False            # set True (e.g. from test.py) to capture NTFF profile
LAST_RESULTS = None        # BassKernelResults of the last run (for test.py)

_built = None


def _to_fp32r(x: np.ndarray) -> np.ndarray:
    """Round fp32 -> fp32r encoding (RNE to 11 explicit mantissa bits)."""
    u = np.ascontiguousarray(x, dtype=np.float32).view(np.uint32).astype(np.uint64)
    u = (u + 0x7FF + ((u >> 12) & 1)) & np.uint64(0xFFFFF000)
    return u.astype(np.uint32).view(np.float32)


def _build():
    global _built
    if _built is not None:
        return _built

    nc = bacc.Bacc("TRN2", target_bir_lowering=False, debug=False)

    embT_d = nc.dram_tensor("embT", [BPC, NCH, 128, S], F32R, kind="ExternalInput").ap()
    # AT[db, da] = A_aug[da, db]; rows 0..1023 in 8 chunks + row 1024 separate
    at_d = nc.dram_tensor("at", [DA, DA], F32R, kind="ExternalInput").ap()
    lrows_d = nc.dram_tensor("lrows", [BPC, 3, S], F32R, kind="ExternalInput").ap()
    rrows_d = nc.dram_tensor("rrows", [BPC, 2, S], F32R, kind="ExternalInput").ap()
    ucol_d = nc.dram_tensor("ucol", [128, NCH], F32, kind="ExternalInput").ap()
    c0_d = nc.dram_tensor("c0", [1, 1], F32, kind="ExternalInput").ap()
    ones_d = nc.dram_tensor("ones", [128, 1], F32R, kind="ExternalInput").ap()
    warm_d = nc.dram_tensor("warm", [128, 128], F32R, kind="ExternalInput").ap()
    onesrow_d = nc.dram_tensor("onesrow", [1, S], F32R, kind="ExternalInput").ap()

    zw_d = nc.dram_tensor("zw", [BPC, 2, 128, NIC], F32, kind="ExternalOutput").ap()
    rout_d = nc.dram_tensor("rout", [BPC, S], F32, kind="ExternalOutput").ap()

    with tile.TileContext(nc) as tc:
        with (
            tc.tile_pool(name="apool", bufs=9) as apool,
            tc.tile_pool(name="spool", bufs=1) as spool,
            tc.tile_pool(name="epool", bufs=16) as epool,
            tc.tile_pool(name="sqpool", bufs=3) as sqpool,
            tc.tile_pool(name="paugpool", bufs=18) as paugpool,
            tc.tile_pool(name="w2pool", bufs=2) as w2pool,
            tc.tile_pool(name="gapool", bufs=2) as gapool,
            tc.tile_pool(name="gwpool", bufs=4) as gwpool,
            tc.tile_pool(name="Epool", bufs=2) as Epool,
            tc.tile_pool(name="scrpool", bufs=1) as scrpool,
            tc.tile_pool(name="tiny", bufs=2) as tiny,
            tc.tile_pool(name="lrpool", bufs=2) as lrpool,
            tc.tile_pool(name="ps", bufs=8, space="PSUM") as ps,
        ):
            # ---- HAM warmup: a tiny tile is the very first DMA (arrives with
            # the small-transfer wave ~5.5us); dummy matmuls on it warm the PE
            # clock to 2.4GHz before the real data lands (~12us). Cold MMs run
            # 2x slower for the first ~3.4us of PE activity otherwise.
            warm_t = spool.tile([128, 128], F32R, tag="warm")
            nc.sync.dma_start(out=warm_t[:], in_=warm_d)
            warm_ps = ps.tile([128, S], F32, tag="ps", name="warm_ps")
            for _ in range(40):
                nc.tensor.matmul(warm_ps[:, 0:128], warm_t[:], warm_t[:],
                                 start=True, stop=True)

            # ---- first chunk pair goes absolutely first (PE start gate),
            # then the tiny loads, then the remaining big chunks interleaved.
            emb_all = [[None] * NCH for _ in range(BPC)]
            at_t = []
            t = apool.tile([128, DA], F32R, tag="a", name="at_0")
            nc.sync.dma_start(out=t[:], in_=at_d[0:128, :])
            at_t.append(t)
            t = epool.tile([128, S], F32R, tag="emb", name="emb0_0")
            nc.sync.dma_start(out=t[:], in_=embT_d[0, 0])
            emb_all[0][0] = t

            ones_col = spool.tile([128, 1], F32R, tag="ones_col")
            nc.sync.dma_start(out=ones_col[:], in_=ones_d)
            onesrow_t = spool.tile([1, S], F32R, tag="onesrow")
            nc.sync.dma_start(out=onesrow_t[:], in_=onesrow_d)
            ucol_t = spool.tile([128, NCH], F32, tag="ucol")
            nc.sync.dma_start(out=ucol_t[:], in_=ucol_d)
            c0_t = spool.tile([1, 1], F32, tag="c0")
            nc.sync.dma_start(out=c0_t[:], in_=c0_d)
            lr_all = []
            for b in range(BPC):
                lr_t = lrpool.tile([3, S], F32R, tag="lr", name=f"lr{b}")
                nc.sync.dma_start(out=lr_t[:], in_=lrows_d[b])
                lr_all.append(lr_t)

            for c in range(1, NCH):
                t = epool.tile([128, S], F32R, tag="emb", name=f"emb0_{c}")
                nc.sync.dma_start(out=t[:], in_=embT_d[0, c])
                emb_all[0][c] = t
                t = apool.tile([128, DA], F32R, tag="a", name=f"at_{c}")
                nc.sync.dma_start(out=t[:], in_=at_d[c * 128 : (c + 1) * 128, :])
                at_t.append(t)

            for b in range(BPC):
                # ---- load this batch's emb
                if b > 0:
                    for c in range(NCH):
                        t = epool.tile([128, S], F32R, tag="emb", name=f"emb{b}_{c}")
                        nc.sync.dma_start(out=t[:], in_=embT_d[b, c])
                        emb_all[b][c] = t
                emb_t = emb_all[b]
                lr_t = lr_all[b]

                # ---- stage 1: P = A_aug @ embaug.T  (db-outer over 8 banks);
                # the ones-row term (u) is folded into the copy bias below.
                st1 = [ps.tile([128, S], F32, tag="ps", name=f"st1_{b}_{da}")
                       for da in range(NCH)]
                for db in range(NCH):
                    for da in range(NCH):
                        nc.tensor.matmul(st1[da][:],
                                         at_t[db][:, da * 128 : (da + 1) * 128],
                                         emb_t[db][:],
                                         start=(db == 0), stop=(db == NCH - 1))
                paug = []
                for da in range(NCH):
                    pt = paugpool.tile([128, S], F32R, tag="paug")
                    if da % 2 == 0:
                        nc.scalar.activation(out=pt[:], in_=st1[da][:],
                                             func=AFT.Identity,
                                             bias=ucol_t[:, da : da + 1], scale=1.0)
                    else:
                        nc.vector.tensor_scalar_add(pt[:], st1[da][:],
                                                    ucol_t[:, da : da + 1])
                    paug.append(pt)
                # P row 1024 (the bq-side rank-1 term); c0 folded into the bias
                prow_ps = ps.tile([1, S], F32, tag="ps")
                for db in range(NCH):
                    nc.tensor.matmul(prow_ps[:], at_t[db][:, D : D + 1],
                                     emb_t[db][:],
                                     start=(db == 0), stop=(db == NCH - 1))
                prow = tiny.tile([1, S], F32R, tag="prow")
                nc.scalar.activation(out=prow[:], in_=prow_ps[:],
                                     func=AFT.Identity, bias=c0_t[:], scale=1.0)
                # rhs rows for the combined mask+prow matmul (K=3):
                # p0 = ones, p1 = m1neg (host), p2 = prow (device)
                rr3 = lrpool.tile([3, S], F32R, tag="rr3")
                nc.sync.dma_start(out=rr3[0:2, :], in_=rrows_d[b])
                nc.sync.dma_start(out=rr3[2:3, :], in_=prow[:])

                # ---- rsq / r / W2
                sqacc = sqpool.tile([128, S], F32R, tag="sqacc", bufs=2)
                sq0 = sqpool.tile([128, S], F32, tag="sq")
                nc.vector.tensor_mul(sq0[:], emb_t[0][:].bitcast(F32),
                                     emb_t[0][:].bitcast(F32))
                for c in range(1, NCH):
                    sq = sqpool.tile([128, S], F32, tag="sq")
                    nc.vector.tensor_mul(sq[:], emb_t[c][:].bitcast(F32),
                                         emb_t[c][:].bitcast(F32))
                    if c < NCH - 1:
                        nc.vector.tensor_add(sq0[:], sq0[:], sq[:])
                    else:
                        nc.vector.tensor_add(sqacc[:], sq0[:], sq[:])
                rsq_ps = ps.tile([1, S], F32, tag="ps")
                nc.tensor.matmul(rsq_ps[:], ones_col[:], sqacc[:],
                                 start=True, stop=True)
                s_row = tiny.tile([1, S], F32, tag="srow")
                nc.scalar.activation(out=s_row[:], in_=rsq_ps[:], func=AFT.Sqrt,
                                     bias=0.0, scale=1.0)
                r_row = tiny.tile([1, S], F32, tag="rrow")
                nc.vector.reciprocal(out=r_row[:], in_=s_row[:])
                nc.sync.dma_start(out=rout_d[b], in_=r_row[:])
                W2 = w2pool.tile([128, S], F32, tag="w2")
                nc.gpsimd.partition_broadcast(W2[:], r_row[0:1, :], channels=128)

                # ---- stage 2: L chunks + masks; per-chunk max
                mx = tiny.tile([128, NIC], F32, tag="mx")
                L_ps = []
                for ic in range(NIC):
                    Lp = ps.tile([128, S], F32, tag="ps", name=f"L_{b}_{ic}")
                    for da in range(NCH):
                        nc.tensor.matmul(Lp[:], emb_t[da][:, ic * 128 : (ic + 1) * 128],
                                         paug[da][:], start=(da == 0), stop=False)
                    nc.tensor.matmul(Lp[:], lr_t[:, ic * 128 : (ic + 1) * 128],
                                     rr3[:], start=False, stop=True)
                    nc.vector.reduce_max(mx[:, ic : ic + 1], Lp[:], axis=AX.X)
                    L_ps.append(Lp)

                # ---- global masked max -> -M in [128,1]
                par = tiny.tile([128, NIC], F32, tag="par")
                nc.gpsimd.partition_all_reduce(par[:], mx[:], channels=128,
                                               reduce_op=bass_isa.ReduceOp.max)
                negm128 = tiny.tile([128, 1], F32, tag="negm128")
                nc.vector.reduce_max(negm128[:], par[:], axis=AX.X, negate=True)

                # ---- gram chunks -> Gw = |G| * r_j
                gw_t = []
                for ic in range(NIC):
                    Gp = ps.tile([128, S], F32, tag="ps", name=f"G_{b}_{ic}")
                    for c in range(NCH):
                        nc.tensor.matmul(Gp[:], emb_t[c][:, ic * 128 : (ic + 1) * 128],
                                         emb_t[c][:], start=(c == 0), stop=(c == NCH - 1))
                    ga = gapool.tile([128, S], F32, tag="ga")
                    nc.scalar.activation(out=ga[:], in_=Gp[:], func=AFT.Abs,
                                         bias=0.0, scale=1.0)
                    gw = gwpool.tile([128, S], F32, tag="gw")
                    nc.vector.tensor_mul(gw[:], ga[:], W2[:])
                    gw_t.append(gw)

                # ---- exp + fused weighted reductions
                zwcols = tiny.tile([128, 2 * NIC], F32, tag="zwc")
                zcols = zwcols[:, 0:NIC]
                wcols = zwcols[:, NIC : 2 * NIC]
                for ic in range(NIC):
                    E = Epool.tile([128, S], F32, tag="E")
                    nc.scalar.activation(out=E[:], in_=L_ps[ic][:], func=AFT.Exp,
                                         bias=negm128[:], scale=1.0,
                                         accum_out=zcols[:, ic : ic + 1])
                    scr = scrpool.tile([128, S], F32, tag="scr")
                    nc.vector.scalar_tensor_tensor(
                        out=scr[:], in0=gw_t[ic][:], scalar=1.0, in1=E[:],
                        op0=ALU.mult, op1=ALU.mult,
                        accum_out=wcols[:, ic : ic + 1])

                nc.sync.dma_start(out=zw_d[b, 0], in_=zcols[:])
                nc.sync.dma_start(out=zw_d[b, 1], in_=wcols[:])

    nc.compile()
    _built = nc
    return nc


def kernel(embeddings, Wq, bq, Wk, bk, attention_masks, token_type_ids):
    global LAST_RESULTS
    nc = _build()

    embeddings = np.ascontiguousarray(np.asarray(embeddings, dtype=np.float32))
    Wq = np.asarray(Wq, dtype=np.float32)
    Wk = np.asarray(Wk, dtype=np.float32)
    bq = np.asarray(bq, dtype=np.float32)
    bk = np.asarray(bk, dtype=np.float32)
    am = np.asarray(attention_masks)
    tt = np.asarray(token_type_ids)

    # host-side layout + constant folding + fp32r rounding
    embT = _to_fp32r(embeddings.transpose(0, 2, 1)).reshape(B, NCH, 128, S)

    Wq64, Wk64 = Wq.astype(np.float64), Wk.astype(np.float64)
    A_aug = np.empty((DA, DA), np.float64)
    A_aug[:D, :D] = Wq64.T @ Wk64                  # A[d,d'] = sum_e Wq[e,d] Wk[e,d']
    A_aug[:D, D] = Wq64.T @ bk.astype(np.float64)   # u
    A_aug[D, :D] = Wk64.T @ bq.astype(np.float64)   # v
    A_aug[D, D] = float(bq.astype(np.float64) @ bk.astype(np.float64))
    AT = _to_fp32r(np.ascontiguousarray(A_aug.T).astype(np.float32))

    tok = am == 1
    m0 = tok & (tt == 0)
    m1 = tok & (tt == 1)
    m0neg = np.where(m0, np.float32(0.0), NEG).astype(np.float32)
    m1neg = np.where(m1, np.float32(0.0), NEG).astype(np.float32)
    ones_row = np.ones((B, 1, S), np.float32)
    lrows = _to_fp32r(np.concatenate([m0neg[:, None, :], ones_row, ones_row], axis=1))
    rrows = _to_fp32r(np.concatenate([ones_row, m1neg[:, None, :]], axis=1))
    ucol = np.ascontiguousarray(
        A_aug[:D, D].astype(np.float32).reshape(NCH, 128).T)        # [128, NCH]
    c0 = np.array([[A_aug[D, D]]], np.float32)

    in_maps = []
    for i in range(NCORES):
        sl = slice(i * BPC, (i + 1) * BPC)
        in_maps.append({
            "embT": np.ascontiguousarray(embT[sl]),
            "at": AT,
            "lrows": np.ascontiguousarray(lrows[sl]),
            "rrows": np.ascontiguousarray(rrows[sl]),
            "ones": np.ones((128, 1), np.float32),
            "onesrow": np.ones((1, S), np.float32),
            "ucol": ucol, "c0": c0,
            "warm": np.ones((128, 128), np.float32),
        })

    res = run_bass_kernel_spmd(nc, in_maps, core_ids=list(range(NCORES)),
                               trace=PROFILE)
    LAST_RESULTS = res

    valid = m0.any(axis=1) & m1.any(axis=1)
    cs = np.zeros(B, np.float64)
    for i in range(NCORES):
        for j in range(BPC):
            b = i * BPC + j
            if not valid[b]:
                continue
            zcols = res.results[i]["zw"][j, 0].astype(np.float64)   # [128, NIC]
            wcols = res.results[i]["zw"][j, 1].astype(np.float64)
            r = res.results[i]["rout"][j].astype(np.float64)        # [S]
            ri = r.reshape(NIC, 128).T                              # [128, NIC]
            z = zcols.sum()
            w = (wcols * ri).sum()
            cs[b] = w / (z + 1e-30)
    return cs.astype(np.float32)


# revision 21
# speedup vs baseline: 1.0405x; 1.0359x over previous
"""Trainium2 Bass kernel for nn_CESAR_24309514895978 (ragged_sequence).

Math (per batch b):
  m0 = (attention_masks==1)&(token_type_ids==0); m1 = (attention_masks==1)&(token_type_ids==1)
  score[i,j] = |emb_n[i] . emb_n[j]|   (L2-normalized embeddings)
  logits[i,j] = (emb@Wq.T+bq)[i] . (emb@Wk.T+bk)[j]
  cs[b] = sum_{valid ij} softmax_flat(logits | pair_mask)[i,j] * score[i,j]

Constant folding (host, once): the projections only enter through
  logits = embaug @ A_aug @ embaug.T,  embaug = [emb, 1],
  A_aug = [[Wq.T@Wk, Wq.T@bk], [bq.T@Wk, bq.bk]]   ((D+1)x(D+1))
so the per-batch device work is two chained matmuls instead of three.

Device, per batch (data-parallel: 2 batches per core x 8 cores, fp32r matmuls):
  - rsq[j] = sum_d emb[j,d]^2 (DVE squares+adds, one ones-matmul); r = 1/sqrt
  - P = A_aug @ embaug.T   (stage 1, 8 PSUM banks, db-outer accumulation)
  - L = embaug.T.T @ P + (-1e30 masks via a K=2 static matmul row-pair)
  - M = masked max (DVE reduces + gpsimd partition_all_reduce)
  - E = exp(L - M) on ACT with accum_out -> Z partial sums
  - W partials = sum_j E * |G| * r_j  (G = gram matmul; |.| on ACT; stt fused)
Host: r_i scaling + final sums + W/Z division (tiny) + input layout/rounding.
"""
import numpy as np

import concourse.bass_isa as bass_isa
import concourse.tile as tile
from concourse import bacc, mybir
from concourse.bass_utils import run_bass_kernel_spmd

B, S, D = 16, 512, 1024
NCORES = 8
BPC = B // NCORES          # batches per core
NCH = D // 128             # 8 contraction chunks
NIC = S // 128             # 4 i-chunks
DA = D + 1                 # augmented dim
NEG = np.float32(-1e30)

F32 = mybir.dt.float32
F32R = mybir.dt.float32r
AFT = mybir.ActivationFunctionType
ALU = mybir.AluOpType
AX = mybir.AxisListType

PROFILE = False            # set True (e.g. from test.py) to capture NTFF profile
LAST_RESULTS = None        # BassKernelResults of the last run (for test.py)

_built = None


def _to_fp32r(x: np.ndarray) -> np.ndarray:
    """Round fp32 -> fp32r encoding (RNE to 11 explicit mantissa bits)."""
    u = np.ascontiguousarray(x, dtype=np.float32).view(np.uint32).astype(np.uint64)
    u = (u + 0x7FF + ((u >> 12) & 1)) & np.uint64(0xFFFFF000)
    return u.astype(np.uint32).view(np.float32)


def _build():
    global _built
    if _built is not None:
        return _built

    nc = bacc.Bacc("TRN2", target_bir_lowering=False, debug=False)

    embT_d = nc.dram_tensor("embT", [BPC, NCH, 128, S], F32R, kind="ExternalInput").ap()
    # AT[db, da] = A_aug[da, db]; rows 0..1023 in 8 chunks + row 1024 separate
    at_d = nc.dram_tensor("at", [DA, DA], F32R, kind="ExternalInput").ap()
    lrows_d = nc.dram_tensor("lrows", [BPC, 3, S], F32R, kind="ExternalInput").ap()
    rrows_d = nc.dram_tensor("rrows", [BPC, 2, S], F32R, kind="ExternalInput").ap()
    ucol_d = nc.dram_tensor("ucol", [128, NCH], F32, kind="ExternalInput").ap()
    c0_d = nc.dram_tensor("c0", [1, 1], F32, kind="ExternalInput").ap()
    ones_d = nc.dram_tensor("ones", [128, 1], F32R, kind="ExternalInput").ap()
    onesrow_d = nc.dram_tensor("onesrow", [1, S], F32R, kind="ExternalInput").ap()

    zw_d = nc.dram_tensor("zw", [BPC, 2, 128, NIC], F32, kind="ExternalOutput").ap()
    rout_d = nc.dram_tensor("rout", [BPC, S], F32, kind="ExternalOutput").ap()

    with tile.TileContext(nc) as tc:
        with (
            tc.tile_pool(name="apool", bufs=9) as apool,
            tc.tile_pool(name="spool", bufs=1) as spool,
            tc.tile_pool(name="epool", bufs=16) as epool,
            tc.tile_pool(name="sqpool", bufs=3) as sqpool,
            tc.tile_pool(name="paugpool", bufs=18) as paugpool,
            tc.tile_pool(name="w2pool", bufs=2) as w2pool,
            tc.tile_pool(name="gapool", bufs=2) as gapool,
            tc.tile_pool(name="gwpool", bufs=4) as gwpool,
            tc.tile_pool(name="Epool", bufs=2) as Epool,
            tc.tile_pool(name="scrpool", bufs=1) as scrpool,
            tc.tile_pool(name="tiny", bufs=2) as tiny,
            tc.tile_pool(name="lrpool", bufs=2) as lrpool,
            tc.tile_pool(name="ps", bufs=8, space="PSUM") as ps,
        ):
            # ---- first chunk pair goes absolutely first (PE start gate),
            # then the tiny loads, then the remaining big chunks interleaved.
            emb_all = [[None] * NCH for _ in range(BPC)]
            at_t = []
            t = apool.tile([128, DA], F32R, tag="a", name="at_0")
            nc.sync.dma_start(out=t[:], in_=at_d[0:128, :])
            at_t.append(t)
            t = epool.tile([128, S], F32R, tag="emb", name="emb0_0")
            nc.sync.dma_start(out=t[:], in_=embT_d[0, 0])
            emb_all[0][0] = t

            ones_col = spool.tile([128, 1], F32R, tag="ones_col")
            nc.sync.dma_start(out=ones_col[:], in_=ones_d)
            onesrow_t = spool.tile([1, S], F32R, tag="onesrow")
            nc.sync.dma_start(out=onesrow_t[:], in_=onesrow_d)
            ucol_t = spool.tile([128, NCH], F32, tag="ucol")
            nc.sync.dma_start(out=ucol_t[:], in_=ucol_d)
            c0_t = spool.tile([1, 1], F32, tag="c0")
            nc.sync.dma_start(out=c0_t[:], in_=c0_d)
            lr_all = []
            for b in range(BPC):
                lr_t = lrpool.tile([3, S], F32R, tag="lr", name=f"lr{b}")
                nc.sync.dma_start(out=lr_t[:], in_=lrows_d[b])
                lr_all.append(lr_t)

            for c in range(1, NCH):
                t = epool.tile([128, S], F32R, tag="emb", name=f"emb0_{c}")
                nc.sync.dma_start(out=t[:], in_=embT_d[0, c])
                emb_all[0][c] = t
                t = apool.tile([128, DA], F32R, tag="a", name=f"at_{c}")
                nc.sync.dma_start(out=t[:], in_=at_d[c * 128 : (c + 1) * 128, :])
                at_t.append(t)

            for b in range(BPC):
                # ---- load this batch's emb
                if b > 0:
                    for c in range(NCH):
                        t = epool.tile([128, S], F32R, tag="emb", name=f"emb{b}_{c}")
                        nc.sync.dma_start(out=t[:], in_=embT_d[b, c])
                        emb_all[b][c] = t
                emb_t = emb_all[b]
                lr_t = lr_all[b]

                # ---- stage 1: P = A_aug @ embaug.T  (db-outer over 8 banks);
                # the ones-row term (u) is folded into the copy bias below.
                st1 = [ps.tile([128, S], F32, tag="ps", name=f"st1_{b}_{da}")
                       for da in range(NCH)]
                for db in range(NCH):
                    for da in range(NCH):
                        nc.tensor.matmul(st1[da][:],
                                         at_t[db][:, da * 128 : (da + 1) * 128],
                                         emb_t[db][:],
                                         start=(db == 0), stop=(db == NCH - 1))
                paug = []
                for da in range(NCH):
                    pt = paugpool.tile([128, S], F32R, tag="paug")
                    if da % 2 == 0:
                        nc.scalar.activation(out=pt[:], in_=st1[da][:],
                                             func=AFT.Identity,
                                             bias=ucol_t[:, da : da + 1], scale=1.0)
                    else:
                        nc.vector.tensor_scalar_add(pt[:], st1[da][:],
                                                    ucol_t[:, da : da + 1])
                    paug.append(pt)
                # P row 1024 (the bq-side rank-1 term); c0 folded into the bias
                prow_ps = ps.tile([1, S], F32, tag="ps")
                for db in range(NCH):
                    nc.tensor.matmul(prow_ps[:], at_t[db][:, D : D + 1],
                                     emb_t[db][:],
                                     start=(db == 0), stop=(db == NCH - 1))
                prow = tiny.tile([1, S], F32R, tag="prow")
                nc.scalar.activation(out=prow[:], in_=prow_ps[:],
                                     func=AFT.Identity, bias=c0_t[:], scale=1.0)
                # rhs rows for the combined mask+prow matmul (K=3):
                # p0 = ones, p1 = m1neg (host), p2 = prow (device)
                rr3 = lrpool.tile([3, S], F32R, tag="rr3")
                nc.sync.dma_start(out=rr3[0:2, :], in_=rrows_d[b])
                nc.sync.dma_start(out=rr3[2:3, :], in_=prow[:])

                # ---- rsq / r / W2
                sqacc = sqpool.tile([128, S], F32R, tag="sqacc", bufs=2)
                sq0 = sqpool.tile([128, S], F32, tag="sq")
                nc.vector.tensor_mul(sq0[:], emb_t[0][:].bitcast(F32),
                                     emb_t[0][:].bitcast(F32))
                for c in range(1, NCH):
                    sq = sqpool.tile([128, S], F32, tag="sq")
                    nc.vector.tensor_mul(sq[:], emb_t[c][:].bitcast(F32),
                                         emb_t[c][:].bitcast(F32))
                    if c < NCH - 1:
                        nc.vector.tensor_add(sq0[:], sq0[:], sq[:])
                    else:
                        nc.vector.tensor_add(sqacc[:], sq0[:], sq[:])
                rsq_ps = ps.tile([1, S], F32, tag="ps")
                nc.tensor.matmul(rsq_ps[:], ones_col[:], sqacc[:],
                                 start=True, stop=True)
                s_row = tiny.tile([1, S], F32, tag="srow")
                nc.scalar.activation(out=s_row[:], in_=rsq_ps[:], func=AFT.Sqrt,
                                     bias=0.0, scale=1.0)
                r_row = tiny.tile([1, S], F32, tag="rrow")
                nc.vector.reciprocal(out=r_row[:], in_=s_row[:])
                nc.sync.dma_start(out=rout_d[b], in_=r_row[:])
                W2 = w2pool.tile([128, S], F32, tag="w2")
                nc.gpsimd.partition_broadcast(W2[:], r_row[0:1, :], channels=128)

                # ---- stage 2: L chunks + masks; per-chunk max
                mx = tiny.tile([128, NIC], F32, tag="mx")
                L_ps = []
                for ic in range(NIC):
                    Lp = ps.tile([128, S], F32, tag="ps", name=f"L_{b}_{ic}")
                    for da in range(NCH):
                        nc.tensor.matmul(Lp[:], emb_t[da][:, ic * 128 : (ic + 1) * 128],
                                         paug[da][:], start=(da == 0), stop=False)
                    nc.tensor.matmul(Lp[:], lr_t[:, ic * 128 : (ic + 1) * 128],
                                     rr3[:], start=False, stop=True)
                    nc.vector.reduce_max(mx[:, ic : ic + 1], Lp[:], axis=AX.X)
                    L_ps.append(Lp)

                # ---- global masked max -> -M in [128,1]
                par = tiny.tile([128, NIC], F32, tag="par")
                nc.gpsimd.partition_all_reduce(par[:], mx[:], channels=128,
                                               reduce_op=bass_isa.ReduceOp.max)
                negm128 = tiny.tile([128, 1], F32, tag="negm128")
                nc.vector.reduce_max(negm128[:], par[:], axis=AX.X, negate=True)

                # ---- gram chunks -> Gw = |G| * r_j
                gw_t = []
                for ic in range(NIC):
                    Gp = ps.tile([128, S], F32, tag="ps", name=f"G_{b}_{ic}")
                    for c in range(NCH):
                        nc.tensor.matmul(Gp[:], emb_t[c][:, ic * 128 : (ic + 1) * 128],
                                         emb_t[c][:], start=(c == 0), stop=(c == NCH - 1))
                    ga = gapool.tile([128, S], F32, tag="ga")
                    nc.scalar.activation(out=ga[:], in_=Gp[:], func=AFT.Abs,
                                         bias=0.0, scale=1.0)
                    gw = gwpool.tile([128, S], F32, tag="gw")
                    nc.vector.tensor_mul(gw[:], ga[:], W2[:])
                    gw_t.append(gw)

                # ---- exp + fused weighted reductions
                zwcols = tiny.tile([128, 2 * NIC], F32, tag="zwc")
                zcols = zwcols[:, 0:NIC]
                wcols = zwcols[:, NIC : 2 * NIC]
                for ic in range(NIC):
                    E = Epool.tile([128, S], F32, tag="E")
                    nc.scalar.activation(out=E[:], in_=L_ps[ic][:], func=AFT.Exp,
                                         bias=negm128[:], scale=1.0,
                                         accum_out=zcols[:, ic : ic + 1])
                    scr = scrpool.tile([128, S], F32, tag="scr")
                    nc.vector.scalar_tensor_tensor(
                        out=scr[:], in0=gw_t[ic][:], scalar=1.0, in1=E[:],
                        op0=ALU.mult, op1=ALU.mult,
                        accum_out=wcols[:, ic : ic + 1])

                nc.sync.dma_start(out=zw_d[b, 0], in_=zcols[:])
                nc.sync.dma_start(out=zw_d[b, 1], in_=wcols[:])

    nc.compile()
    _built = nc
    return nc


def kernel(embeddings, Wq, bq, Wk, bk, attention_masks, token_type_ids):
    global LAST_RESULTS
    nc = _build()

    embeddings = np.ascontiguousarray(np.asarray(embeddings, dtype=np.float32))
    Wq = np.asarray(Wq, dtype=np.float32)
    Wk = np.asarray(Wk, dtype=np.float32)
    bq = np.asarray(bq, dtype=np.float32)
    bk = np.asarray(bk, dtype=np.float32)
    am = np.asarray(attention_masks)
    tt = np.asarray(token_type_ids)

    # host-side layout + constant folding + fp32r rounding
    embT = _to_fp32r(embeddings.transpose(0, 2, 1)).reshape(B, NCH, 128, S)

    Wq64, Wk64 = Wq.astype(np.float64), Wk.astype(np.float64)
    A_aug = np.empty((DA, DA), np.float64)
    A_aug[:D, :D] = Wq64.T @ Wk64                  # A[d,d'] = sum_e Wq[e,d] Wk[e,d']
    A_aug[:D, D] = Wq64.T @ bk.astype(np.float64)   # u
    A_aug[D, :D] = Wk64.T @ bq.astype(np.float64)   # v
    A_aug[D, D] = float(bq.astype(np.float64) @ bk.astype(np.float64))
    AT = _to_fp32r(np.ascontiguousarray(A_aug.T).astype(np.float32))

    tok = am == 1
    m0 = tok & (tt == 0)
    m1 = tok & (tt == 1)
    m0neg = np.where(m0, np.float32(0.0), NEG).astype(np.float32)
    m1neg = np.where(m1, np.float32(0.0), NEG).astype(np.float32)
    ones_row = np.ones((B, 1, S), np.float32)
    lrows = _to_fp32r(np.concatenate([m0neg[:, None, :], ones_row, ones_row], axis=1))
    rrows = _to_fp32r(np.concatenate([ones_row, m1neg[:, None, :]], axis=1))
    ucol = np.ascontiguousarray(
        A_aug[:D, D].astype(np.float32).reshape(NCH, 128).T)        # [128, NCH]
    c0 = np.array([[A_aug[D, D]]], np.float32)

    in_maps = []
    for i in range(NCORES):
        sl = slice(i * BPC, (i + 1) * BPC)
        in_maps.append({
            "embT": np.ascontiguousarray(embT[sl]),
            "at": AT,
            "lrows": np.ascontiguousarray(lrows[sl]),
            "rrows": np.ascontiguousarray(rrows[sl]),
            "ones": np.ones((128, 1), np.float32),
            "onesrow": np.ones((1, S), np.float32),
            "ucol": ucol, "c0": c0,
        })

    res = run_bass_kernel_spmd(nc, in_maps, core_ids=list(range(NCORES)),
                               trace=PROFILE)
    LAST_RESULTS = res

    valid = m0.any(axis=1) & m1.any(axis=1)
    cs = np.zeros(B, np.float64)
    for i in range(NCORES):
        for j in range(BPC):
            b = i * BPC + j
            if not valid[b]:
                continue
            zcols = res.results[i]["zw"][j, 0].astype(np.float64)   # [128, NIC]
            wcols = res.results[i]["zw"][j, 1].astype(np.float64)
            r = res.results[i]["rout"][j].astype(np.float64)        # [S]
            ri = r.reshape(NIC, 128).T                              # [128, NIC]
            z = zcols.sum()
            w = (wcols * ri).sum()
            cs[b] = w / (z + 1e-30)
    return cs.astype(np.float32)
